# revision 14
# baseline (speedup 1.0000x reference)
"""Trainium2 Bass kernel for nn_LocalitySelfAttention (causal self-attention).

Math (per batch element b):
    qkv = x @ w_qkv ; split q,k,v into 16 heads of d=64
    dots = (q @ k^T) * scale_h ; mask strictly lower (j < i allowed)
    attn = softmax(dots) with fully-masked rows -> 0
    out  = concat_h(attn @ v) @ w_out + b_out

Sharding: data-parallel over batch B=8 across the 8 NeuronCores (weights
replicated). Each core computes one full batch element.

Device algorithm (per core), all matmuls in bf16 (PE 1 cycle/row and
lower power than the fp32 path, which DVFS-throttled the baseline):

  One flat instruction stream; projections, attention and the output
  projection are interleaved so the in-order PE never waits for the
  ScalarE exp stream.  Work is organized as "chains" (8 accumulating
  matmuls into one PSUM bank + a copy-out) used as filler between the
  attention dot-product pairs:
    - startup: v (natural layout + ones column) for key blocks 0..3,
      q^T/k^T head-chunk 0 (query half 0).
    - nb=0 query block, head pairs hp=0..7: dots^T as K=64 co-run
      pairs (PE row groups 0-63/64-127), exp on ScalarE -> bf16, diag
      mask multiply on Pool; fillers: q^T/k^T chunk hp+1 (half 0) and
      v blocks 4..7; then attn^T @ v_aug accumulation (ones column
      gives the softmax denominator as psum row 64).
    - nb=1 likewise; fillers: q^T/k^T half 1 and the nb=0 output
      projection; tail: nb=1 output projection.
  Denominator chain per head pair: guard+extract psum row 64 into
  [1, 2*NB] (DVE), two accumulating K=1 matmuls broadcast the pair into
  [128, NB] (rows 0-63 head0, 64-127 head1) via a [1, 256] selector,
  one DVE reciprocal, one DVE multiply against the ScalarE-evacuated
  attn-out pair.
  No row-max subtraction: logits for this problem are bounded well
  inside fp32 exp range; fully-masked query 0 yields 0 (flash-attn
  convention, matching the reference).
"""

import os
import sys

for _p in ("/opt/trn_rl_repo", "/root/.axon_site/_ro/trn_rl_repo"):
    if os.path.isdir(_p) and _p not in sys.path:
        sys.path.append(_p)

import ml_dtypes
import numpy as np

import concourse.bass as bass  # noqa: F401  (AP helpers)
import concourse.mybir as mybir
import concourse.tile as tile
from concourse import bacc
from concourse.bass_utils import run_bass_kernel_spmd

F = mybir.dt.float32
R = mybir.dt.float32r
BF = mybir.dt.bfloat16

B, N, C, H = 8, 1024, 1024, 16
D = C // H          # 64
NB = 512            # query block (free dim of attention matmuls)
KC = C // 128       # 8 contraction chunks
HP = H // 2         # 8 head pairs == c' chunks
NCORES = 8

_cache: dict = {}


def _build():
    nc = bacc.Bacc("TRN2", target_bir_lowering=False, debug=False,
                   num_devices=NCORES)
    xT_d = nc.dram_tensor("xT", [C, N], BF, kind="ExternalInput")
    wq_d = nc.dram_tensor("wq", [C, C], BF, kind="ExternalInput")
    wk_d = nc.dram_tensor("wk", [C, C], BF, kind="ExternalInput")
    wv_d = nc.dram_tensor("wv", [C, C], BF, kind="ExternalInput")
    wout_d = nc.dram_tensor("wout", [C, C], BF, kind="ExternalInput")
    boutr_d = nc.dram_tensor("boutr", [128, C], F, kind="ExternalInput")
    sclq_d = nc.dram_tensor("sclq", [128, HP], F, kind="ExternalInput")
    masku_d = nc.dram_tensor("masku", [128, 128], BF, kind="ExternalInput")
    sel2_d = nc.dram_tensor("sel2", [1, 256], F, kind="ExternalInput")
    onesb_d = nc.dram_tensor("onesb", [128, H], BF, kind="ExternalInput")
    y_d = nc.dram_tensor("y", [N, C], F, kind="ExternalOutput")

    with tile.TileContext(nc) as tc:
        with (
            tc.tile_pool(name="const", bufs=1) as cp,
            tc.tile_pool(name="persist", bufs=1) as pp,
            tc.tile_pool(name="psum_d", bufs=2, space="PSUM") as ps_d,
            tc.tile_pool(name="psum_o", bufs=1, space="PSUM") as ps_o,
            tc.tile_pool(name="psum_mm", bufs=2, space="PSUM") as ps_mm,
            tc.tile_pool(name="atp", bufs=8) as ap,
            tc.tile_pool(name="evacp", bufs=2) as ep,
            tc.tile_pool(name="denp", bufs=2) as dp,
            tc.tile_pool(name="rdenp", bufs=2) as rp,
            tc.tile_pool(name="aop", bufs=2) as aop,
            tc.tile_pool(name="outp", bufs=2) as op,
        ):
            # ---- persistent tiles ----
            xt = [pp.tile([128, N], BF, tag=f"x{i}", name=f"xt{i}")
                  for i in range(KC)]
            wqt = [pp.tile([128, C], BF, tag=f"wq{i}", name=f"wqt{i}")
                   for i in range(KC)]
            wkt = [pp.tile([128, C], BF, tag=f"wk{i}", name=f"wkt{i}")
                   for i in range(KC)]
            wvt = [pp.tile([128, C], BF, tag=f"wv{i}", name=f"wvt{i}")
                   for i in range(KC)]
            wot = [pp.tile([128, C], BF, tag=f"wo{i}", name=f"wot{i}")
                   for i in range(KC)]
            qT = [pp.tile([128, N], BF, tag=f"qT{i}", name=f"qT{i}")
                  for i in range(KC)]
            kT = [pp.tile([128, N], BF, tag=f"kT{i}", name=f"kT{i}")
                  for i in range(KC)]
            vaug = [pp.tile([128, H * (D + 1)], BF, tag=f"va{i}",
                            name=f"va{i}")
                    for i in range(KC)]

            # ---- input DMAs (order matters: V path first) ----
            for i in range(KC):
                nc.sync.dma_start(out=xt[i][:],
                                  in_=xT_d[i * 128:(i + 1) * 128, :])
                nc.sync.dma_start(out=wvt[i][:],
                                  in_=wv_d[i * 128:(i + 1) * 128, :])
            for i in range(KC):
                nc.sync.dma_start(out=wqt[i][:],
                                  in_=wq_d[i * 128:(i + 1) * 128, :])
            for i in range(KC):
                nc.sync.dma_start(out=wkt[i][:],
                                  in_=wk_d[i * 128:(i + 1) * 128, :])
            for i in range(KC):
                nc.sync.dma_start(out=wot[i][:],
                                  in_=wout_d[i * 128:(i + 1) * 128, :])

            # constants on the gpsimd DMA queue
            sclq = cp.tile([128, HP], F, name="sclq")
            nc.gpsimd.dma_start(out=sclq[:], in_=sclq_d[:, :])
            masku = cp.tile([128, 128], BF, name="masku")
            nc.gpsimd.dma_start(out=masku[:], in_=masku_d[:, :])
            boutr = cp.tile([128, C], F, name="boutr")
            nc.gpsimd.dma_start(out=boutr[:], in_=boutr_d[:, :])
            sel2 = cp.tile([1, 256], R, name="sel2")
            nc.gpsimd.dma_start(out=sel2[:], in_=sel2_d[:, :].bitcast(R))
            ones_view = onesb_d[:, 0:H].rearrange("p (h o) -> p h o", o=1)

            # ---- filler chains (8 matmuls into one PSUM bank + copy) ----
            def v_chain(m, cvb):
                ps = ps_mm.tile([128, NB], F, tag="mm",
                                name=f"ps_v_{m}_{cvb}")
                for kc in range(KC):
                    nc.tensor.matmul(
                        ps[:],
                        xt[kc][:, m * 128:(m + 1) * 128],
                        wvt[kc][:, cvb * NB:(cvb + 1) * NB],
                        start=(kc == 0), stop=(kc == KC - 1))
                hpb = NB // D             # heads per 512 block = 8
                dst = vaug[m].rearrange(
                    "p (h c) -> p h c",
                    c=D + 1)[:, cvb * hpb:(cvb + 1) * hpb, 0:D]
                src = ps[:].rearrange("p (h c) -> p h c", c=D)
                if cvb == 0:
                    nc.scalar.copy(out=dst, in_=src)
                else:
                    nc.vector.tensor_copy(out=dst, in_=src)
                    va_ones = vaug[m].rearrange(
                        "p (h c) -> p h c", c=D + 1)[:, :, D:D + 1]
                    nc.gpsimd.dma_start(out=va_ones, in_=ones_view)

            def qk_chain(which, m, nbh):
                wch, dst = ((wqt, qT), (wkt, kT))[which]
                ps = ps_mm.tile([128, NB], F, tag="mm",
                                name=f"ps_{which}_{m}_{nbh}")
                for kc in range(KC):
                    nc.tensor.matmul(
                        ps[:],
                        wch[kc][:, m * 128:(m + 1) * 128],
                        xt[kc][:, nbh * NB:(nbh + 1) * NB],
                        start=(kc == 0), stop=(kc == KC - 1))
                if which == 0:
                    nc.vector.tensor_scalar_mul(
                        out=dst[m][:, nbh * NB:(nbh + 1) * NB],
                        in0=ps[:], scalar1=sclq[:, m:m + 1])
                else:
                    nc.vector.tensor_copy(
                        out=dst[m][:, nbh * NB:(nbh + 1) * NB],
                        in_=ps[:])

            def outproj_chain(nb, nsl, cb, ao, outt):
                ps = ps_mm.tile([128, NB], F, tag="mm",
                                name=f"psf_{nb}_{nsl}_{cb}")
                for ci in range(KC):
                    nc.tensor.matmul(
                        ps[:],
                        ao[ci][:, nsl * 128:(nsl + 1) * 128],
                        wot[ci][:, cb * NB:(cb + 1) * NB],
                        start=(ci == 0), stop=(ci == KC - 1))
                nc.vector.tensor_add(
                    out=outt[:, cb * NB:(cb + 1) * NB],
                    in0=ps[:],
                    in1=boutr[:, cb * NB:(cb + 1) * NB])
                if cb == C // NB - 1:
                    row = nb * NB + nsl * 128
                    nc.sync.dma_start(out=y_d[row:row + 128, :],
                                      in_=outt[:])

            # ---- attention pieces ----
            def attn_front(nb, hp, jbmax, fillers):
                """dots + exp + mask; fillers woven into the psd stream."""
                fill_after = {1, 3, 6} if jbmax == 8 else {1}
                ats = []
                for jb in range(jbmax):
                    off = jb * 128 - nb * NB
                    s = max(off, 0)
                    psd = ps_d.tile([128, 2 * NB], F, tag="d",
                                    name=f"psd_{nb}_{hp}_{jb}")
                    for g in range(2):
                        po_p = g * 64
                        nc.tensor.matmul(
                            psd[:, g * NB + s:(g + 1) * NB],
                            kT[hp][po_p:po_p + 64,
                                   jb * 128:(jb + 1) * 128],
                            qT[hp][po_p:po_p + 64,
                                   nb * NB + s:(nb + 1) * NB],
                            start=True, stop=True)
                    atp = ap.tile([128, 2 * NB], BF, tag="at",
                                  name=f"at_{nb}_{hp}_{jb}")
                    nc.scalar.activation(
                        out=atp.rearrange(
                            "p (g n) -> p g n", n=NB)[:, :, s:NB],
                        in_=psd.rearrange(
                            "p (g n) -> p g n", n=NB)[:, :, s:NB],
                        func=mybir.ActivationFunctionType.Exp)
                    if off >= 0:
                        for g in range(2):
                            nc.gpsimd.tensor_mul(
                                out=atp[:, g * NB + s:g * NB + s + 128],
                                in0=atp[:, g * NB + s:g * NB + s + 128],
                                in1=masku[:, :])
                    ats.append((atp, s))
                    if jb in fill_after and fillers:
                        fillers.pop(0)()
                return ats

            def attn_back(nb, hp, jbmax, ats):
                """attn^T @ v_aug + denominator extract for (nb, hp)."""
                po = ps_o.tile([D + 1, 2 * NB], F, tag="po",
                               name=f"po_{nb}_{hp}")
                for jb in range(jbmax):
                    atp, s = ats[jb]
                    for g in range(2):
                        h = 2 * hp + g
                        nc.tensor.matmul(
                            po[:, g * NB + s:(g + 1) * NB],
                            vaug[jb][:, h * (D + 1):(h + 1) * (D + 1)],
                            atp[:, g * NB + s:(g + 1) * NB],
                            start=(jb == 0), stop=(jb == jbmax - 1))
                den2 = dp.tile([1, 2 * NB], R, tag="den",
                               name=f"den_{nb}_{hp}")
                evac = ep.tile([128, NB], F, tag="ev", name=f"ev_{nb}_{hp}")
                nc.vector.tensor_scalar_max(
                    out=den2[0:1, :], in0=po[D:D + 1, :], scalar1=1e-30)
                for g in range(2):
                    nc.scalar.copy(
                        out=evac[g * 64:(g + 1) * 64, :],
                        in_=po[0:D, g * NB:(g + 1) * NB])
                return den2, evac

            def den_finish(nb, hp, den2, evac, ao):
                """broadcast den to both heads, reciprocal, scale out."""
                bc = ps_mm.tile([128, NB], F, tag="mm",
                                name=f"bc_{nb}_{hp}")
                for g in range(2):
                    nc.tensor.matmul(
                        bc[:],
                        sel2[0:1, g * 128:(g + 1) * 128],
                        den2[0:1, g * NB:(g + 1) * NB],
                        start=(g == 0), stop=(g == 1))
                rden = rp.tile([128, NB], F, tag="rden",
                               name=f"rden_{nb}_{hp}")
                nc.vector.reciprocal_approx_fast(out=rden[:], in_=bc[:])
                nc.vector.tensor_mul(out=ao[hp][:], in0=evac[:],
                                     in1=rden[:])

            # ---- schedule ----
            for m in range(4):
                v_chain(m, 0)
                v_chain(m, 1)
            qk_chain(0, 0, 0)
            qk_chain(1, 0, 0)

            ao0 = [aop.tile([128, NB], BF, tag=f"ao{ci}", name=f"ao0_{ci}")
                   for ci in range(KC)]
            ao1 = [aop.tile([128, NB], BF, tag=f"ao{ci}", name=f"ao1_{ci}")
                   for ci in range(KC)]
            outt0 = [op.tile([128, C], F, tag="out", name=f"outt0_{i}")
                     for i in range(NB // 128)]

            pend = None          # (nb, hp, den2, evac, ao) awaiting finish
            for hp in range(HP):
                fillers = []
                if hp < 7:
                    fillers.append(lambda m=hp + 1: qk_chain(0, m, 0))
                    fillers.append(lambda m=hp + 1: qk_chain(1, m, 0))
                else:
                    fillers.append(lambda: qk_chain(0, 0, 1))
                    fillers.append(lambda: qk_chain(1, 0, 1))
                fillers.append(lambda m=4 + hp // 2, c=hp % 2: v_chain(m, c))
                ats = attn_front(0, hp, 4, fillers)
                if pend is not None:
                    den_finish(*pend)
                for f in fillers:
                    f()
                del fillers[:]
                pend = (0, hp, *attn_back(0, hp, 4, ats), ao0)

            op_sched = {1: [(0, 0)], 2: [(0, 1)], 3: [(1, 0)],
                        4: [(1, 1)], 5: [(2, 0), (2, 1)],
                        6: [(3, 0)], 7: [(3, 1)]}
            for hp in range(HP):
                fillers = []
                if hp < 7:
                    fillers.append(lambda m=hp + 1: qk_chain(0, m, 1))
                    fillers.append(lambda m=hp + 1: qk_chain(1, m, 1))
                for nsl, cb in op_sched.get(hp, []):
                    fillers.append(
                        lambda n=nsl, c=cb: outproj_chain(0, n, c, ao0,
                                                          outt0[n]))
                ats = attn_front(1, hp, 8, fillers)
                if pend is not None:
                    den_finish(*pend)
                for f in fillers:
                    f()
                del fillers[:]
                pend = (1, hp, *attn_back(1, hp, 8, ats), ao1)
            den_finish(*pend)

            outt1 = [op.tile([128, C], F, tag="out", name=f"outt1_{i}")
                     for i in range(NB // 128)]
            for nsl in range(NB // 128):
                for cb in range(C // NB):
                    outproj_chain(1, nsl, cb, ao1, outt1[nsl])

    nc.compile()
    return nc


def _get_nc():
    if "nc" not in _cache:
        _cache["nc"] = _build()
    return _cache["nc"]


def _make_in_maps(x, w_qkv, scale, w_out, b_out):
    bf = ml_dtypes.bfloat16
    wq = np.ascontiguousarray(w_qkv[:, 0:C]).astype(bf)
    wk = np.ascontiguousarray(w_qkv[:, C:2 * C]).astype(bf)
    wv = np.ascontiguousarray(w_qkv[:, 2 * C:3 * C]).astype(bf)
    wout = np.ascontiguousarray(w_out).astype(bf)
    boutr = np.ascontiguousarray(
        np.broadcast_to(np.asarray(b_out, np.float32).reshape(1, C),
                        (128, C)))
    sc = np.asarray(scale, np.float32).reshape(H)
    sclq = np.empty((128, HP), np.float32)
    sclq[0:64, :] = sc[0::2][None, :]
    sclq[64:128, :] = sc[1::2][None, :]
    masku = np.triu(np.ones((128, 128), np.float32), k=1).astype(bf)
    sel2 = np.zeros((1, 256), np.float32)
    sel2[0, 0:64] = 1.0
    sel2[0, 192:256] = 1.0
    onesb = np.ones((128, H), bf)
    maps = []
    for b in range(NCORES):
        xT = np.ascontiguousarray(np.asarray(x[b], np.float32).T).astype(bf)
        maps.append({"xT": xT, "wq": wq, "wk": wk, "wv": wv, "wout": wout,
                     "boutr": boutr, "sclq": sclq, "masku": masku,
                     "sel2": sel2, "onesb": onesb})
    return maps


def _run(x, w_qkv, scale, w_out, b_out, trace=False, tmpdir=None):
    nc = _get_nc()
    in_maps = _make_in_maps(x, w_qkv, scale, w_out, b_out)
    res = run_bass_kernel_spmd(nc, in_maps, list(range(NCORES)),
                               trace=trace, tmpdir=tmpdir)
    out = np.stack([res.results[i]["y"] for i in range(NCORES)], axis=0)
    return out.astype(np.float32), res


def kernel(x, w_qkv, scale, w_out, b_out):
    out, _ = _run(x, w_qkv, scale, w_out, b_out)
    return out


# revision 18
# speedup vs baseline: 1.0138x; 1.0138x over previous
"""Trainium2 Bass kernel for nn_LocalitySelfAttention (causal self-attention).

Math (per batch element b):
    qkv = x @ w_qkv ; split q,k,v into 16 heads of d=64
    dots = (q @ k^T) * scale_h ; mask strictly lower (j < i allowed)
    attn = softmax(dots) with fully-masked rows -> 0
    out  = concat_h(attn @ v) @ w_out + b_out

Sharding: data-parallel over batch B=8 across the 8 NeuronCores (weights
replicated). Each core computes one full batch element.

Device algorithm (per core), all matmuls in bf16 (PE 1 cycle/row and
lower power than the fp32 path, which DVFS-throttled the baseline):

  One flat instruction stream; projections, attention and the output
  projection are interleaved so the in-order PE never waits for the
  ScalarE exp stream.  Work is organized as "chains" (8 accumulating
  matmuls into one PSUM bank + a copy-out) used as filler between the
  attention dot-product pairs:
    - startup: v (natural layout + ones column) for key blocks 0..3,
      q^T/k^T head-chunk 0 (query half 0).
    - nb=0 query block, head pairs hp=0..7: dots^T as K=64 co-run
      pairs (PE row groups 0-63/64-127), exp on ScalarE -> bf16, diag
      mask multiply on Pool; fillers: q^T/k^T chunk hp+1 (half 0) and
      v blocks 4..7; then attn^T @ v_aug accumulation (ones column
      gives the softmax denominator as psum row 64).
    - nb=1 likewise; fillers: q^T/k^T half 1 and the nb=0 output
      projection; tail: nb=1 output projection.
  Denominator chain per head pair: guard+extract psum row 64 into
  [1, 2*NB] (DVE), two accumulating K=1 matmuls broadcast the pair into
  [128, NB] (rows 0-63 head0, 64-127 head1) via a [1, 256] selector,
  one DVE reciprocal, one DVE multiply against the ScalarE-evacuated
  attn-out pair.
  No row-max subtraction: logits for this problem are bounded well
  inside fp32 exp range; fully-masked query 0 yields 0 (flash-attn
  convention, matching the reference).
"""

import os
import sys

for _p in ("/opt/trn_rl_repo", "/root/.axon_site/_ro/trn_rl_repo"):
    if os.path.isdir(_p) and _p not in sys.path:
        sys.path.append(_p)

import ml_dtypes
import numpy as np

import concourse.bass as bass  # noqa: F401  (AP helpers)
import concourse.mybir as mybir
import concourse.tile as tile
from concourse import bacc
from concourse.bass_utils import run_bass_kernel_spmd

F = mybir.dt.float32
R = mybir.dt.float32r
BF = mybir.dt.bfloat16

B, N, C, H = 8, 1024, 1024, 16
D = C // H          # 64
NB = 512            # query block (free dim of attention matmuls)
KC = C // 128       # 8 contraction chunks
HP = H // 2         # 8 head pairs == c' chunks
NCORES = 8

_cache: dict = {}


def _build():
    nc = bacc.Bacc("TRN2", target_bir_lowering=False, debug=False,
                   num_devices=NCORES)
    xT_d = nc.dram_tensor("xT", [C, N], BF, kind="ExternalInput")
    wq_d = nc.dram_tensor("wq", [C, C], BF, kind="ExternalInput")
    wk_d = nc.dram_tensor("wk", [C, C], BF, kind="ExternalInput")
    wv_d = nc.dram_tensor("wv", [C, C], BF, kind="ExternalInput")
    wout_d = nc.dram_tensor("wout", [C, C], BF, kind="ExternalInput")
    boutr_d = nc.dram_tensor("boutr", [128, C], F, kind="ExternalInput")
    masku_d = nc.dram_tensor("masku", [128, 128], BF, kind="ExternalInput")
    sel2_d = nc.dram_tensor("sel2", [1, 256], F, kind="ExternalInput")
    onesb_d = nc.dram_tensor("onesb", [128, H], BF, kind="ExternalInput")
    y_d = nc.dram_tensor("y", [N, C], F, kind="ExternalOutput")

    with tile.TileContext(nc) as tc:
        with (
            tc.tile_pool(name="const", bufs=1) as cp,
            tc.tile_pool(name="persist", bufs=1) as pp,
            tc.tile_pool(name="psum_d", bufs=2, space="PSUM") as ps_d,
            tc.tile_pool(name="psum_o", bufs=1, space="PSUM") as ps_o,
            tc.tile_pool(name="psum_mm", bufs=2, space="PSUM") as ps_mm,
            tc.tile_pool(name="atp", bufs=8) as ap,
            tc.tile_pool(name="evacp", bufs=2) as ep,
            tc.tile_pool(name="denp", bufs=2) as dp,
            tc.tile_pool(name="rdenp", bufs=2) as rp,
            tc.tile_pool(name="aop", bufs=2) as aop,
            tc.tile_pool(name="outp", bufs=2) as op,
            tc.tile_pool(name="splp", bufs=1) as sp,
        ):
            # ---- persistent tiles ----
            xt = [pp.tile([128, N], BF, tag=f"x{i}", name=f"xt{i}")
                  for i in range(KC)]
            wqt = [pp.tile([128, C], BF, tag=f"wq{i}", name=f"wqt{i}")
                   for i in range(KC)]
            wkt = [pp.tile([128, C], BF, tag=f"wk{i}", name=f"wkt{i}")
                   for i in range(KC)]
            wvt = [pp.tile([128, C], BF, tag=f"wv{i}", name=f"wvt{i}")
                   for i in range(KC)]
            wot = [pp.tile([128, C], BF, tag=f"wo{i}", name=f"wot{i}")
                   for i in range(KC)]
            qT = [pp.tile([128, N], BF, tag=f"qT{i}", name=f"qT{i}")
                  for i in range(KC)]
            kT = [pp.tile([128, N], BF, tag=f"kT{i}", name=f"kT{i}")
                  for i in range(KC)]
            vaug = [pp.tile([128, H * (D + 1)], BF, tag=f"va{i}",
                            name=f"va{i}")
                    for i in range(KC)]

            # ---- input DMAs (order matters: V path first) ----
            for i in range(KC):
                nc.sync.dma_start(out=xt[i][:],
                                  in_=xT_d[i * 128:(i + 1) * 128, :])
                nc.sync.dma_start(out=wvt[i][:],
                                  in_=wv_d[i * 128:(i + 1) * 128, :])
            for i in range(KC):
                nc.sync.dma_start(out=wqt[i][:],
                                  in_=wq_d[i * 128:(i + 1) * 128, :])
            for i in range(KC):
                nc.sync.dma_start(out=wkt[i][:],
                                  in_=wk_d[i * 128:(i + 1) * 128, :])
            for i in range(KC):
                nc.sync.dma_start(out=wot[i][:],
                                  in_=wout_d[i * 128:(i + 1) * 128, :])

            # constants on the gpsimd DMA queue
            masku = cp.tile([128, 128], BF, name="masku")
            nc.gpsimd.dma_start(out=masku[:], in_=masku_d[:, :])
            boutr = cp.tile([128, C], F, name="boutr")
            nc.gpsimd.dma_start(out=boutr[:], in_=boutr_d[:, :])
            sel2 = cp.tile([1, 256], R, name="sel2")
            nc.gpsimd.dma_start(out=sel2[:], in_=sel2_d[:, :].bitcast(R))
            ones_view = onesb_d[:, 0:H].rearrange("p (h o) -> p h o", o=1)

            # ---- filler chains (8 matmuls into one PSUM bank + copy) ----
            def v_chain(m, cvb):
                ps = ps_mm.tile([128, NB], F, tag="mm",
                                name=f"ps_v_{m}_{cvb}")
                for kc in range(KC):
                    nc.tensor.matmul(
                        ps[:],
                        xt[kc][:, m * 128:(m + 1) * 128],
                        wvt[kc][:, cvb * NB:(cvb + 1) * NB],
                        start=(kc == 0), stop=(kc == KC - 1))
                hpb = NB // D             # heads per 512 block = 8
                dst = vaug[m].rearrange(
                    "p (h c) -> p h c",
                    c=D + 1)[:, cvb * hpb:(cvb + 1) * hpb, 0:D]
                src = ps[:].rearrange("p (h c) -> p h c", c=D)
                if cvb == 0:
                    nc.scalar.copy(out=dst, in_=src)
                else:
                    nc.vector.tensor_copy(out=dst, in_=src)
                    va_ones = vaug[m].rearrange(
                        "p (h c) -> p h c", c=D + 1)[:, :, D:D + 1]
                    nc.gpsimd.dma_start(out=va_ones, in_=ones_view)

            def qk_chain(which, m, nbh):
                wch, dst = ((wqt, qT), (wkt, kT))[which]
                ps = ps_mm.tile([128, NB], F, tag="mm",
                                name=f"ps_{which}_{m}_{nbh}")
                for kc in range(KC):
                    nc.tensor.matmul(
                        ps[:],
                        wch[kc][:, m * 128:(m + 1) * 128],
                        xt[kc][:, nbh * NB:(nbh + 1) * NB],
                        start=(kc == 0), stop=(kc == KC - 1))
                nc.vector.tensor_copy(
                    out=dst[m][:, nbh * NB:(nbh + 1) * NB],
                    in_=ps[:])

            def outproj_chain(nb, nsl, cb, ao, outt):
                ps = ps_mm.tile([128, NB], F, tag="mm",
                                name=f"psf_{nb}_{nsl}_{cb}")
                for ci in range(KC):
                    nc.tensor.matmul(
                        ps[:],
                        ao[ci][:, nsl * 128:(nsl + 1) * 128],
                        wot[ci][:, cb * NB:(cb + 1) * NB],
                        start=(ci == 0), stop=(ci == KC - 1))
                nc.vector.tensor_add(
                    out=outt[:, cb * NB:(cb + 1) * NB],
                    in0=ps[:],
                    in1=boutr[:, cb * NB:(cb + 1) * NB])
                row = nb * NB + nsl * 128
                nc.sync.dma_start(
                    out=y_d[row:row + 128, cb * NB:(cb + 1) * NB],
                    in_=outt[:, cb * NB:(cb + 1) * NB])

            # ---- attention pieces ----
            def attn_front(nb, hp, jbmax, fillers):
                """dots + exp + mask; fillers woven into the psd stream."""
                fill_after = {1, 3, 6} if jbmax == 8 else {1}
                ats = []
                for jb in range(jbmax):
                    off = jb * 128 - nb * NB
                    s = max(off, 0)
                    psd = ps_d.tile([128, 2 * NB], F, tag="d",
                                    name=f"psd_{nb}_{hp}_{jb}")
                    for g in range(2):
                        po_p = g * 64
                        nc.tensor.matmul(
                            psd[:, g * NB + s:(g + 1) * NB],
                            kT[hp][po_p:po_p + 64,
                                   jb * 128:(jb + 1) * 128],
                            qT[hp][po_p:po_p + 64,
                                   nb * NB + s:(nb + 1) * NB],
                            start=True, stop=True)
                    atp = ap.tile([128, 2 * NB], BF, tag="at",
                                  name=f"at_{nb}_{hp}_{jb}")
                    nc.scalar.activation(
                        out=atp.rearrange(
                            "p (g n) -> p g n", n=NB)[:, :, s:NB],
                        in_=psd.rearrange(
                            "p (g n) -> p g n", n=NB)[:, :, s:NB],
                        func=mybir.ActivationFunctionType.Exp)
                    if off >= 0:
                        for g in range(2):
                            nc.gpsimd.tensor_mul(
                                out=atp[:, g * NB + s:g * NB + s + 128],
                                in0=atp[:, g * NB + s:g * NB + s + 128],
                                in1=masku[:, :])
                    ats.append((atp, s))
                    if jb in fill_after and fillers:
                        fillers.pop(0)()
                return ats

            def attn_back(nb, hp, jbmax, ats):
                """attn^T @ v_aug + denominator extract for (nb, hp)."""
                po = ps_o.tile([D + 1, 2 * NB], F, tag="po",
                               name=f"po_{nb}_{hp}")
                for jb in range(jbmax):
                    atp, s = ats[jb]
                    for g in range(2):
                        h = 2 * hp + g
                        nc.tensor.matmul(
                            po[:, g * NB + s:(g + 1) * NB],
                            vaug[jb][:, h * (D + 1):(h + 1) * (D + 1)],
                            atp[:, g * NB + s:(g + 1) * NB],
                            start=(jb == 0), stop=(jb == jbmax - 1))
                den2 = dp.tile([1, 2 * NB], R, tag="den",
                               name=f"den_{nb}_{hp}")
                evac = ep.tile([128, NB], F, tag="ev", name=f"ev_{nb}_{hp}")
                nc.vector.tensor_scalar_max(
                    out=den2[0:1, :], in0=po[D:D + 1, :], scalar1=1e-30)
                for g in range(2):
                    nc.scalar.copy(
                        out=evac[g * 64:(g + 1) * 64, :],
                        in_=po[0:D, g * NB:(g + 1) * NB])
                return den2, evac

            def den_finish(nb, hp, den2, evac, ao):
                """broadcast den to both heads, reciprocal, scale out."""
                bc = ps_mm.tile([128, NB], F, tag="mm",
                                name=f"bc_{nb}_{hp}")
                for g in range(2):
                    nc.tensor.matmul(
                        bc[:],
                        sel2[0:1, g * 128:(g + 1) * 128],
                        den2[0:1, g * NB:(g + 1) * NB],
                        start=(g == 0), stop=(g == 1))
                rden = rp.tile([128, NB], F, tag="rden",
                               name=f"rden_{nb}_{hp}")
                nc.vector.reciprocal_approx_fast(out=rden[:], in_=bc[:])
                nc.vector.tensor_mul(out=ao[hp][:], in0=evac[:],
                                     in1=rden[:])

            # ---- schedule ----
            # startup: split each chain into kc 0..3 / kc 4..7 halves so
            # the PE starts as soon as half of x/wv has streamed in; the
            # A-half evacuates to SBUF, the B-half combines on the copy.
            units = ([("v", m, cvb) for m in range(4) for cvb in range(2)]
                     + [("qk", 0, 0), ("qk", 1, 0)])
            tmps = []
            for idx, (kind, a1, a2) in enumerate(units):
                psA = ps_mm.tile([128, NB], F, tag="mm",
                                 name=f"spA_{idx}")
                for kc in range(4):
                    if kind == "v":
                        nc.tensor.matmul(
                            psA[:], xt[kc][:, a1 * 128:(a1 + 1) * 128],
                            wvt[kc][:, a2 * NB:(a2 + 1) * NB],
                            start=(kc == 0), stop=(kc == 3))
                    else:
                        wch = (wqt, wkt)[a1]
                        nc.tensor.matmul(
                            psA[:], wch[kc][:, 0:128],
                            xt[kc][:, 0:NB],
                            start=(kc == 0), stop=(kc == 3))
                tmp = sp.tile([128, NB], BF, tag=f"sp{idx}",
                              name=f"sp_{idx}")
                if idx % 2 == 0:
                    nc.scalar.copy(out=tmp[:], in_=psA[:])
                else:
                    nc.vector.tensor_copy(out=tmp[:], in_=psA[:])
                tmps.append(tmp)
            for idx, (kind, a1, a2) in enumerate(units):
                psB = ps_mm.tile([128, NB], F, tag="mm",
                                 name=f"spB_{idx}")
                for kc in range(4, KC):
                    if kind == "v":
                        nc.tensor.matmul(
                            psB[:], xt[kc][:, a1 * 128:(a1 + 1) * 128],
                            wvt[kc][:, a2 * NB:(a2 + 1) * NB],
                            start=(kc == 4), stop=(kc == KC - 1))
                    else:
                        wch = (wqt, wkt)[a1]
                        nc.tensor.matmul(
                            psB[:], wch[kc][:, 0:128],
                            xt[kc][:, 0:NB],
                            start=(kc == 4), stop=(kc == KC - 1))
                tmp = tmps[idx]
                if kind == "v":
                    m, cvb = a1, a2
                    hpb = NB // D
                    dst = vaug[m].rearrange(
                        "p (h c) -> p h c",
                        c=D + 1)[:, cvb * hpb:(cvb + 1) * hpb, 0:D]
                    nc.vector.tensor_add(
                        out=dst, in0=psB[:].rearrange("p (h c) -> p h c", c=D),
                        in1=tmp[:].rearrange("p (h c) -> p h c", c=D))
                    if cvb == 1:
                        va_ones = vaug[m].rearrange(
                            "p (h c) -> p h c", c=D + 1)[:, :, D:D + 1]
                        nc.gpsimd.dma_start(out=va_ones, in_=ones_view)
                else:
                    dstt = (qT, kT)[a1]
                    nc.vector.tensor_add(out=dstt[0][:, 0:NB],
                                         in0=psB[:], in1=tmp[:])

            ao0 = [aop.tile([128, NB], BF, tag=f"ao{ci}", name=f"ao0_{ci}")
                   for ci in range(KC)]
            ao1 = [aop.tile([128, NB], BF, tag=f"ao{ci}", name=f"ao1_{ci}")
                   for ci in range(KC)]
            outt0 = [op.tile([128, C], F, tag="out", name=f"outt0_{i}")
                     for i in range(NB // 128)]

            pend = None          # (nb, hp, den2, evac, ao) awaiting finish
            for hp in range(HP):
                fillers = []
                if hp < 7:
                    fillers.append(lambda m=hp + 1: qk_chain(0, m, 0))
                    fillers.append(lambda m=hp + 1: qk_chain(1, m, 0))
                else:
                    fillers.append(lambda: qk_chain(0, 0, 1))
                    fillers.append(lambda: qk_chain(1, 0, 1))
                fillers.append(lambda m=4 + hp // 2, c=hp % 2: v_chain(m, c))
                ats = attn_front(0, hp, 4, fillers)
                if pend is not None:
                    den_finish(*pend)
                for f in fillers:
                    f()
                del fillers[:]
                pend = (0, hp, *attn_back(0, hp, 4, ats), ao0)

            op_sched = {1: [(0, 0)], 2: [(0, 1)], 3: [(1, 0)],
                        4: [(1, 1)], 5: [(2, 0), (2, 1)],
                        6: [(3, 0)], 7: [(3, 1)]}
            for hp in range(HP):
                fillers = []
                if hp < 7:
                    fillers.append(lambda m=hp + 1: qk_chain(0, m, 1))
                    fillers.append(lambda m=hp + 1: qk_chain(1, m, 1))
                for nsl, cb in op_sched.get(hp, []):
                    fillers.append(
                        lambda n=nsl, c=cb: outproj_chain(0, n, c, ao0,
                                                          outt0[n]))
                ats = attn_front(1, hp, 8, fillers)
                if pend is not None:
                    den_finish(*pend)
                for f in fillers:
                    f()
                del fillers[:]
                pend = (1, hp, *attn_back(1, hp, 8, ats), ao1)
            den_finish(*pend)

            outt1 = [op.tile([128, C], F, tag="out", name=f"outt1_{i}")
                     for i in range(NB // 128)]
            for nsl in range(NB // 128):
                for cb in range(C // NB):
                    outproj_chain(1, nsl, cb, ao1, outt1[nsl])

    nc.compile()
    return nc


def _get_nc():
    if "nc" not in _cache:
        _cache["nc"] = _build()
    return _cache["nc"]


def _make_in_maps(x, w_qkv, scale, w_out, b_out):
    bf = ml_dtypes.bfloat16
    sc_col = np.repeat(np.asarray(scale, np.float32).reshape(H), D)
    wq = np.ascontiguousarray(w_qkv[:, 0:C] * sc_col[None, :]).astype(bf)
    wk = np.ascontiguousarray(w_qkv[:, C:2 * C]).astype(bf)
    wv = np.ascontiguousarray(w_qkv[:, 2 * C:3 * C]).astype(bf)
    wout = np.ascontiguousarray(w_out).astype(bf)
    boutr = np.ascontiguousarray(
        np.broadcast_to(np.asarray(b_out, np.float32).reshape(1, C),
                        (128, C)))
    masku = np.triu(np.ones((128, 128), np.float32), k=1).astype(bf)
    sel2 = np.zeros((1, 256), np.float32)
    sel2[0, 0:64] = 1.0
    sel2[0, 192:256] = 1.0
    onesb = np.ones((128, H), bf)
    maps = []
    for b in range(NCORES):
        xT = np.ascontiguousarray(np.asarray(x[b], np.float32).T).astype(bf)
        maps.append({"xT": xT, "wq": wq, "wk": wk, "wv": wv, "wout": wout,
                     "boutr": boutr, "masku": masku,
                     "sel2": sel2, "onesb": onesb})
    return maps


def _run(x, w_qkv, scale, w_out, b_out, trace=False, tmpdir=None):
    nc = _get_nc()
    in_maps = _make_in_maps(x, w_qkv, scale, w_out, b_out)
    res = run_bass_kernel_spmd(nc, in_maps, list(range(NCORES)),
                               trace=trace, tmpdir=tmpdir)
    out = np.stack([res.results[i]["y"] for i in range(NCORES)], axis=0)
    return out.astype(np.float32), res


def kernel(x, w_qkv, scale, w_out, b_out):
    out, _ = _run(x, w_qkv, scale, w_out, b_out)
    return out
